# revision 2
# baseline (speedup 1.0000x reference)
"""Trainium2 Bass kernel for nn_DecoderPreLN2 (4-layer cross-attention decoder).

Sharding: data-parallel over batch N=8 across 8 NeuronCores (1 element/core).
Per-core dataflow is fully "transposed" (D-major): activations live as x^T
[D=1024 partitions(8 tiles), T free] so every matmul keeps weights stationary
and no on-device transposes are needed.

Key tricks:
  - fp32r (fp32 rounded to 11 mantissa bits) matmuls: 4x faster than fp32.
  - LayerNorm g/b and the 1/sqrt(HD) scale folded into wq/bq host-side.
  - V-projection bias bv folded into the NEXT layer's LN stats (softmax
    weights sum to 1, so +bv commutes through attention); last layer adds
    bv explicitly.
  - Scores computed transposed [k, q]; softmax denominator via a ones-column
    appended to V (row 64 of the AV psum = sum of exp weights); the
    cross_attn_mask enters as the exp's per-partition bias; no max
    subtraction (logits are O(1) by construction).
  - Softmax normalize: DVE reciprocal + GPSIMD partition_broadcast + one
    DVE multiply.
"""

import sys

sys.path.insert(0, "/opt/trn_rl_repo")

import numpy as np

import concourse.bass as bass
import concourse.tile as tile
from concourse import bacc, mybir
from concourse.bass import ts
from concourse.bass_utils import run_bass_kernel_spmd

L, D, H, HD = 4, 1024, 16, 64
TQ, TK, NB = 512, 1024, 8
DT = D // 128  # 8 d-tiles
KT = TK // 128  # 8 k-token tiles
EPS = 1e-5

F32 = mybir.dt.float32
F32R = mybir.dt.float32r
AF = mybir.ActivationFunctionType
OP = mybir.AluOpType


def round_fp32r(a: np.ndarray) -> np.ndarray:
    """Round-to-nearest-even to fp32r (11 explicit mantissa bits)."""
    u = np.ascontiguousarray(a, dtype=np.float32).view(np.uint32).astype(np.uint64)
    bias = np.uint64(0x7FF) + ((u >> np.uint64(12)) & np.uint64(1))
    u = (u + bias) & np.uint64(0xFFFFF000)
    return u.astype(np.uint32).view(np.float32)


_PROGRAM = None


def build_program():
    global _PROGRAM
    if _PROGRAM is not None:
        return _PROGRAM

    nc = bacc.Bacc(
        "TRN2", target_bir_lowering=False, debug=False,
        dynamic_dma_scratch_size=2048,
    )

    xt0 = nc.dram_tensor("xt0", [D, TQ], F32R, kind="ExternalInput").ap()
    enct = nc.dram_tensor("enct", [D, TK], F32R, kind="ExternalInput").ap()
    maskd = nc.dram_tensor("maskd", [128, KT], F32, kind="ExternalInput").ap()
    wqd = nc.dram_tensor("wqd", [L, D, D], F32R, kind="ExternalInput").ap()
    wkd = nc.dram_tensor("wkd", [L, D, D], F32R, kind="ExternalInput").ap()
    wvd = nc.dram_tensor("wvd", [L, D, D], F32R, kind="ExternalInput").ap()
    bqd = nc.dram_tensor("bqd", [L, 128, DT], F32, kind="ExternalInput").ap()
    bkd = nc.dram_tensor("bkd", [L, 128, DT], F32, kind="ExternalInput").ap()
    bvfd = nc.dram_tensor("bvfd", [L, 128, DT], F32, kind="ExternalInput").ap()
    bvpd = nc.dram_tensor("bvpd", [L, 128, DT], F32, kind="ExternalInput").ap()
    bvp2d = nc.dram_tensor("bvp2d", [L, 128, DT], F32R, kind="ExternalInput").ap()
    lncd = nc.dram_tensor("lncd", [L, 1, 2], F32, kind="ExternalInput").ap()
    onesd = nc.dram_tensor("onesd", [128, 1], F32R, kind="ExternalInput").ap()
    outd = nc.dram_tensor("outd", [D, TQ], F32, kind="ExternalOutput").ap()

    with tile.TileContext(nc) as tc:
        with (
            tc.tile_pool(name="persist", bufs=1) as persist,
            tc.tile_pool(name="xp", bufs=2) as xp,
            tc.tile_pool(name="wp", bufs=18) as wp,
            tc.tile_pool(name="sqp", bufs=3) as sqp,
            tc.tile_pool(name="ptp", bufs=4) as ptp,
            tc.tile_pool(name="bias", bufs=2) as biasp,
            tc.tile_pool(name="smalls", bufs=1) as smalls,
            tc.tile_pool(name="recipp", bufs=3) as recipp,
            tc.tile_pool(name="bcp", bufs=2) as bcp,
            tc.tile_pool(name="proj_ps", bufs=2, space="PSUM") as proj_ps,
            tc.tile_pool(name="stat_ps", bufs=1, space="PSUM") as stat_ps,
            tc.tile_pool(name="sc_ps", bufs=3, space="PSUM") as sc_ps,
            tc.tile_pool(name="av_ps", bufs=2, space="PSUM") as av_ps,
        ):
            # ---- persistent tiles ----
            enc_sb = persist.tile([128, DT, TK], F32R, tag="enc")
            nc.sync.dma_start(
                out=enc_sb[:], in_=enct.rearrange("(j p) t -> p j t", p=128)
            )
            kt_sb = persist.tile([128, DT, TK], F32R, tag="kt")
            qt_sb = persist.tile([128, DT, TQ], F32R, tag="qt")
            vaug_sb = persist.tile([128, KT, H, HD + 1], F32R, tag="vaug")
            ones_sb = persist.tile([128, 1], F32R, tag="ones")
            nc.sync.dma_start(out=ones_sb[:], in_=onesd[:])
            mask_sb = persist.tile([128, KT], F32, tag="mask")
            nc.sync.dma_start(out=mask_sb[:], in_=maskd[:])
            onesf_sb = persist.tile([128, H], F32, tag="onesf")
            nc.vector.memset(onesf_sb[:], 1.0)
            # ones column of v_aug: written once, never overwritten after
            for m in range(KT):
                nc.vector.tensor_copy(vaug_sb[:, m, :, HD : HD + 1], onesf_sb[:])
            rstd_bc = persist.tile([128, TQ], F32, tag="rstd_bc")
            murstd_bc = persist.tile([128, TQ], F32, tag="murstd_bc")

            # layer-0 x
            x_cur = xp.tile([128, DT, TQ], F32R, tag="x")
            nc.sync.dma_start(
                out=x_cur[:], in_=xt0.rearrange("(j p) t -> p j t", p=128)
            )

            def load_w_half(wd, i, half):
                """Load 8 [128, 512] half-tiles (din-tile k, dout half)."""
                tiles = []
                for k in range(DT):
                    w = wp.tile([128, 512], F32R, tag="w")
                    nc.sync.dma_start(
                        out=w[:], in_=wd[i, ts(k, 128), ts(half, 512)]
                    )
                    tiles.append(w)
                return tiles

            for i in range(L):
                # ---- per-layer constant loads ----
                bq_sb = biasp.tile([128, DT], F32, tag="bq")
                nc.sync.dma_start(out=bq_sb[:], in_=bqd[i])
                bk_sb = biasp.tile([128, DT], F32, tag="bk")
                nc.sync.dma_start(out=bk_sb[:], in_=bkd[i])
                bvp_sb = biasp.tile([128, DT], F32, tag="bvp")
                nc.sync.dma_start(out=bvp_sb[:], in_=bvpd[i])
                bvp2_sb = biasp.tile([128, DT], F32R, tag="bvp2")
                nc.sync.dma_start(out=bvp2_sb[:], in_=bvp2d[i])
                lnc_sb = biasp.tile([1, 2], F32, tag="lnc")
                nc.sync.dma_start(out=lnc_sb[:], in_=lncd[i])
                if i == L - 1:
                    bvf_sb = biasp.tile([128, DT], F32, tag="bvf")
                    nc.sync.dma_start(out=bvf_sb[:], in_=bvfd[i])

                # ---- LN statistics (reduction over partitions, via PE) ----
                sums_ps = stat_ps.tile([1, TQ], F32, tag="stat")
                for k in range(DT):
                    nc.tensor.matmul(
                        sums_ps[:], ones_sb[:], x_cur[:, k, :],
                        start=(k == 0), stop=(k == DT - 1),
                    )
                sq_tiles = []
                for k in range(DT):
                    sq = sqp.tile([128, TQ], F32R, tag="sq")
                    nc.vector.tensor_mul(sq[:], x_cur[:, k, :], x_cur[:, k, :])
                    sq_tiles.append(sq)
                m2_ps = stat_ps.tile([1, TQ], F32, tag="stat")
                for k in range(DT):
                    nc.tensor.matmul(
                        m2_ps[:], ones_sb[:], sq_tiles[k][:],
                        start=(k == 0), stop=False,
                    )
                # cross term: sum_d 2*bv_prev[d]*x[d,t], same psum group
                for k in range(DT):
                    nc.tensor.matmul(
                        m2_ps[:], bvp2_sb[:, k : k + 1], x_cur[:, k, :],
                        start=False, stop=(k == DT - 1),
                    )

                # ---- K projection: kT[dout, tk] = wk.T @ encT ----
                for half in range(2):
                    wk_t = load_w_half(wkd, i, half)
                    for nl in range(4):
                        n = half * 4 + nl
                        for c in range(2):
                            ps = proj_ps.tile([128, 512], F32, tag="proj")
                            for k in range(DT):
                                nc.tensor.matmul(
                                    ps[:], wk_t[k][:, ts(nl, 128)],
                                    enc_sb[:, k, ts(c, 512)],
                                    start=(k == 0), stop=(k == DT - 1),
                                )
                            nc.vector.tensor_scalar_add(
                                kt_sb[:, n, ts(c, 512)], ps[:],
                                bk_sb[:, n : n + 1],
                            )

                # ---- LN smalls (DVE/ACT, hidden under K/V projections) ----
                mu = smalls.tile([1, TQ], F32, tag="mu")
                nc.vector.tensor_scalar(
                    mu[:], sums_ps[:], lnc_sb[0:1, 0:1], 1.0 / D,
                    op0=OP.add, op1=OP.mult,
                )
                veps = smalls.tile([1, TQ], F32, tag="veps")
                nc.vector.tensor_scalar(
                    veps[:], m2_ps[:], lnc_sb[0:1, 1:2], 1.0 / D,
                    op0=OP.add, op1=OP.mult,
                )
                musq = smalls.tile([1, TQ], F32, tag="musq")
                nc.vector.tensor_mul(musq[:], mu[:], mu[:])
                # veps = (musq * -1) + veps + EPS  (in place)
                nc.vector.scalar_tensor_tensor(
                    veps[:], musq[:], -1.0, veps[:], op0=OP.mult, op1=OP.add
                )
                nc.vector.tensor_scalar_add(veps[:], veps[:], EPS)
                y0 = smalls.tile([1, TQ], F32, tag="y0")
                nc.scalar.activation(y0[:], veps[:], AF.Sqrt)
                nc.vector.reciprocal(y0[:], y0[:])
                # one Newton step: rstd = y0 * (1.5 - 0.5*veps*y0^2)
                t1 = smalls.tile([1, TQ], F32, tag="t1")
                nc.vector.tensor_mul(t1[:], y0[:], y0[:])
                nc.vector.scalar_tensor_tensor(
                    t1[:], t1[:], -0.5, veps[:], op0=OP.mult, op1=OP.mult
                )
                nc.vector.tensor_scalar_add(t1[:], t1[:], 1.5)
                nc.vector.tensor_mul(t1[:], t1[:], y0[:])  # t1 = rstd
                # mu = -mu * rstd  (in place)
                nc.vector.scalar_tensor_tensor(
                    mu[:], mu[:], -1.0, t1[:], op0=OP.mult, op1=OP.mult
                )
                nc.gpsimd.partition_broadcast(rstd_bc[:], t1[:])
                nc.gpsimd.partition_broadcast(murstd_bc[:], mu[:])

                # ---- V projection: v[tk, dout] = encT.T @ wv ----
                for half in range(2):
                    wv_t = load_w_half(wvd, i, half)
                    for m in range(KT):
                        ps = proj_ps.tile([128, 512], F32, tag="proj")
                        for k in range(DT):
                            nc.tensor.matmul(
                                ps[:], enc_sb[:, k, ts(m, 128)], wv_t[k][:],
                                start=(k == 0), stop=(k == DT - 1),
                            )
                        nc.vector.tensor_copy(
                            vaug_sb[:, m, ts(half, 8), 0:HD],
                            ps[:].rearrange("p (h e) -> p h e", h=8),
                        )

                # ---- xn = ((x + bv_prev) - mu) * rstd  (fp32r) ----
                xn = xp.tile([128, DT, TQ], F32R, tag="x")
                for k in range(DT):
                    nc.vector.scalar_tensor_tensor(
                        xn[:, k, :], x_cur[:, k, :], bvp_sb[:, k : k + 1],
                        rstd_bc[:], op0=OP.add, op1=OP.mult,
                    )
                    nc.vector.tensor_tensor(
                        xn[:, k, :], xn[:, k, :], murstd_bc[:], op=OP.add
                    )

                # ---- Q projection: qT[dout, tq] = wq.T @ xn ----
                for half in range(2):
                    wq_t = load_w_half(wqd, i, half)
                    for nl in range(4):
                        n = half * 4 + nl
                        ps = proj_ps.tile([128, 512], F32, tag="proj")
                        for k in range(DT):
                            nc.tensor.matmul(
                                ps[:], wq_t[k][:, ts(nl, 128)], xn[:, k, :],
                                start=(k == 0), stop=(k == DT - 1),
                            )
                        nc.vector.tensor_scalar_add(
                            qt_sb[:, n, :], ps[:], bq_sb[:, n : n + 1]
                        )

                # ---- attention, head by head ----
                x_next = xp.tile([128, DT, TQ], F32R, tag="x")
                for h in range(H):
                    j, o = h // 2, (h % 2) * 64
                    pts = []
                    for kt in range(KT):
                        sc = sc_ps.tile([128, TQ], F32, tag="sc")
                        nc.tensor.matmul(
                            sc[:],
                            kt_sb[o : o + 64, j, ts(kt, 128)],
                            qt_sb[o : o + 64, j, :],
                            start=True, stop=True,
                        )
                        pt = ptp.tile([128, TQ], F32R, tag="pt")
                        nc.scalar.activation(
                            pt[:], sc[:], AF.Exp,
                            bias=mask_sb[:, kt : kt + 1], scale=1.0,
                        )
                        pts.append(pt)
                    av = av_ps.tile([HD + 1, TQ], F32, tag="av")
                    for kt in range(KT):
                        nc.tensor.matmul(
                            av[:], vaug_sb[:, kt, h, :], pts[kt][:],
                            start=(kt == 0), stop=(kt == KT - 1),
                        )
                    recip = recipp.tile([1, TQ], F32, tag="recip")
                    nc.vector.reciprocal(recip[:], av[HD : HD + 1, :])
                    bc = bcp.tile([64, TQ], F32, tag="bc")
                    nc.gpsimd.partition_broadcast(bc[:], recip[:])
                    nc.vector.tensor_tensor(
                        x_next[o : o + 64, j, :], av[0:HD, :], bc[:], op=OP.mult
                    )
                    if i == L - 1:
                        nc.vector.tensor_scalar_add(
                            x_next[o : o + 64, j, :],
                            x_next[o : o + 64, j, :],
                            bvf_sb[o : o + 64, j : j + 1],
                        )
                x_cur = x_next

            nc.sync.dma_start(
                out=outd.rearrange("(j p) t -> p j t", p=128),
                in_=x_cur[:].bitcast(F32),
            )

    nc.compile()
    _PROGRAM = nc
    return nc


def _stage_inputs(input_ids, encoder_state, cross_attn_mask, emb,
                  ln_g, ln_b, wq, bq, wk, bk, wv, bv):
    input_ids = np.asarray(input_ids)
    emb = np.asarray(emb, dtype=np.float32)
    encoder_state = np.asarray(encoder_state, dtype=np.float32)
    cross_attn_mask = np.asarray(cross_attn_mask, dtype=np.float32)
    ln_g = np.asarray(ln_g, dtype=np.float32)
    ln_b = np.asarray(ln_b, dtype=np.float32)
    wq = np.asarray(wq, dtype=np.float32)
    bq = np.asarray(bq, dtype=np.float32)
    wk = np.asarray(wk, dtype=np.float32)
    bk = np.asarray(bk, dtype=np.float32)
    wv = np.asarray(wv, dtype=np.float32)
    bv = np.asarray(bv, dtype=np.float32)

    scale = 1.0 / np.sqrt(HD)
    # fold LN affine + scores scale into wq/bq
    wq2 = ln_g[:, :, None] * wq * scale  # [L, D, D]
    bq2 = (np.einsum("ld,lde->le", ln_b, wq) + bq) * scale  # [L, D]

    def pcol(a):  # [L, D] -> [L, 128, DT] with a[l, j*128+p] at [l, p, j]
        return np.ascontiguousarray(a.reshape(L, DT, 128).transpose(0, 2, 1))

    bv_prev = np.concatenate([np.zeros((1, D), np.float32), bv[:-1]], axis=0)
    lnc = np.stack(
        [bv_prev.sum(axis=1), (bv_prev * bv_prev).sum(axis=1)], axis=1
    ).astype(np.float32)[:, None, :]  # [L, 1, 2]

    shared = {
        "wqd": round_fp32r(wq2),
        "wkd": round_fp32r(wk),
        "wvd": round_fp32r(wv),
        "bqd": pcol(bq2),
        "bkd": pcol(bk),
        "bvfd": pcol(bv),
        "bvpd": pcol(bv_prev),
        "bvp2d": round_fp32r(pcol(2.0 * bv_prev)),
        "lncd": np.ascontiguousarray(lnc),
        "onesd": np.ones((128, 1), np.float32),
    }

    x0 = emb[input_ids]  # [NB, TQ, D]
    in_maps = []
    for n in range(NB):
        m = dict(shared)
        m["xt0"] = round_fp32r(x0[n].T)
        m["enct"] = round_fp32r(encoder_state[n].T)
        m["maskd"] = np.ascontiguousarray(
            cross_attn_mask[n, 0, 0].reshape(KT, 128).T
        )
        in_maps.append(m)
    return in_maps


def kernel(**inputs) -> np.ndarray:
    nc = build_program()
    in_maps = _stage_inputs(**inputs)
    res = run_bass_kernel_spmd(nc, in_maps, list(range(NB)))
    out = np.stack([np.asarray(res.results[n]["outd"]).T for n in range(NB)])
    return np.ascontiguousarray(out, dtype=np.float32)


if __name__ == "__main__":
    build_program()
    print("program built ok")
